# revision 14
# baseline (speedup 1.0000x reference)
"""Trainium2 Bass kernel for nn_AlternateConvolution (gnn_message_passing).

Computation (reference):
    w  = (H_e @ p.T)[:, 0]                    # [NE]
    M1 = (T * w) @ T.T                        # [NV, NV]  (symmetric)
    A  = (eye + (1-eye)*M1) * adj_v
    ret = A @ (H_v @ weight) + bias           # [NV, OUT_V]
    return (ret, H_e)

Distribution: output-row shard over 8 NeuronCores (256 rows each); each core
streams the full T^T in bf16 (no collectives). Per-core inputs are column-
rolled so each core's own 256 columns sit at offset 0 of every k-tile; the
scaled stationary operand is sliced straight out of the streamed k-tiles.
The big GEMM keeps the 256-row slab stationary and streams T^T k-tiles
through the moving port (N=512), so LDWEIGHTS traffic stays tiny.
"""

import os
import sys
import types

import numpy as np
import ml_dtypes

NV, NE = 2048, 8192
IN_V, OUT_V, IN_E = 128, 128, 128
NCORES = 8
R = NV // NCORES          # 256 output rows per core
JT = NV // 128            # 16 j-tiles
KO = NE // 128            # 64 k-tiles
HCH = 8                   # chunks for head-critical DMAs / w pipeline
CH = KO // HCH

BF16 = ml_dtypes.bfloat16

_cache = {}
last_exec_time_ns = None
last_results = None


def _ensure_ntff_hook():
    """Register the NTFF profiling hook if the image's antenv lacks it."""
    try:
        import antenv  # noqa: F401
        import antenv.axon_hooks  # noqa: F401
        return
    except ImportError:
        pass
    try:
        import antenv
        from trn_agent_boot.trn_boot import _ntff_profile_via_ctypes

        hook = _ntff_profile_via_ctypes("/opt/axon/libaxon_pjrt.so")
        mod = types.ModuleType("antenv.axon_hooks")
        mod.get_axon_ntff_profile_hook = lambda: hook
        mod.set_axon_ntff_profile_hook = lambda h: None
        sys.modules["antenv.axon_hooks"] = mod
        antenv.axon_hooks = mod
    except Exception:
        pass


def _build():
    import concourse.mybir as mybir
    import concourse.tile as tile
    from concourse import bacc

    F32 = mybir.dt.float32
    B16 = mybir.dt.bfloat16

    nc = bacc.Bacc("TRN2", target_bir_lowering=False, debug=False,
                   num_devices=NCORES)

    # T^T with columns rolled by -256*core: kt[:, 0:R] is this core's slab.
    TTr = nc.declare_dram_parameter("TTr", [KO, 128, NV], B16, isOutput=False)
    Hen = nc.declare_dram_parameter("Hen", [128, KO, 128], B16, isOutput=False)
    pb = nc.declare_dram_parameter("pb", [128, 128], B16, isOutput=False)
    adjR = nc.declare_dram_parameter("adjR", [128, 2, NV], F32, isOutput=False)
    HvT = nc.declare_dram_parameter("HvT", [128, JT, 128], B16, isOutput=False)
    HvTr = nc.declare_dram_parameter("HvTr", [128, R], B16, isOutput=False)
    Wp = nc.declare_dram_parameter("W", [128, 128], B16, isOutput=False)
    dvalsb = nc.declare_dram_parameter("dvalsb", [128, R], F32, isOutput=False)
    biasP = nc.declare_dram_parameter("biasP", [128, 1], F32, isOutput=False)
    eye = nc.declare_dram_parameter("eye", [128, 128], B16, isOutput=False)
    # transposed output: out[f, r] = ret[rows_c[r], f]; host transposes back
    out = nc.declare_dram_parameter("out", [OUT_V, R], F32, isOutput=True)

    with tile.TileContext(nc) as tc:
        with (
            tc.tile_pool(name="pers", bufs=1) as pers,
            tc.tile_pool(name="ktp", bufs=10) as ktp,
            tc.tile_pool(name="amtp", bufs=3) as amtp,
            tc.tile_pool(name="wscr", bufs=3) as wscr,
            tc.tile_pool(name="PS", bufs=8, space="PSUM") as PS,
        ):
            # ---- head-critical DMAs (everything else is emitted later) ----
            pb_sb = pers.tile([128, 128], B16)
            nc.sync.dma_start(pb_sb[:], pb[:])
            Hen_sb = pers.tile([128, KO, 128], B16)
            for i in range(HCH):
                sl = slice(i * CH, (i + 1) * CH)
                nc.sync.dma_start(Hen_sb[:, sl, :], Hen[:, sl, :])

            HvT_sb = pers.tile([128, JT, 128], B16)
            nc.sync.dma_start(HvT_sb[:], HvT[:])
            HvTr_sb = pers.tile([128, R], B16)
            nc.sync.dma_start(HvTr_sb[:], HvTr[:])
            W_sb = pers.tile([128, 128], B16)
            nc.sync.dma_start(W_sb[:], Wp[:])

            w_sb = pers.tile([128, KO], F32)
            G_sb = pers.tile([128, JT, 128], B16)
            scaled_sb = pers.tile([128, KO, R], B16)


            # ---- G = H_v @ weight in the head (DVE is idle here) ----
            for jt in range(JT):
                g_ps = PS.tile([128, 512], mybir.dt.float32, tag="bank")
                nc.tensor.matmul(g_ps[:, :128], lhsT=HvT_sb[:, jt, :],
                                 rhs=W_sb[:], start=True, stop=True)
                nc.vector.tensor_copy(G_sb[:, jt, :], g_ps[:, :128])
            # G_rows^T for the diagonal term: [f, r]
            grT_sb = pers.tile([128, R], F32)
            grT_ps = PS.tile([128, 512], mybir.dt.float32, tag="bank",
                             name="grT_ps")
            nc.tensor.matmul(grT_ps[:, :R], lhsT=W_sb[:], rhs=HvTr_sb[:],
                             start=True, stop=True)
            nc.vector.tensor_copy(grT_sb[:], grT_ps[:, :R])

            # ---- GEMM1: psA[rh, jc] += scaled[:,ko,rh]^T @ TT[ko][:, jc] ----
            psA = [PS.tile([128, 512], mybir.dt.float32, tag="bank",
                           name=f"psA_{i}") for i in range(8)]
            for ko in range(KO):
                kt = ktp.tile([128, NV], B16, tag="kt")
                nc.sync.dma_start(kt[:], TTr[ko])
                # w[k-tile] on the VectorEngine, just in time for the scale
                wsc = wscr.tile([128, 128], F32, tag="wsc")
                nc.vector.tensor_tensor(wsc[:], Hen_sb[:, ko, :], pb_sb[:],
                                        mybir.AluOpType.mult)
                nc.vector.reduce_sum(w_sb[:, ko : ko + 1], wsc[:],
                                     axis=mybir.AxisListType.X)
                # scaled slab slice straight from the streamed k-tile
                nc.vector.tensor_scalar_mul(scaled_sb[:, ko, :], kt[:, :R],
                                            w_sb[:, ko : ko + 1])
                for rh in range(2):
                    for jc in range(4):
                        nc.tensor.matmul(
                            psA[rh * 4 + jc][:],
                            lhsT=scaled_sb[:, ko, rh * 128 : (rh + 1) * 128],
                            rhs=kt[:, jc * 512 : (jc + 1) * 512],
                            start=(ko == 0), stop=(ko == KO - 1))
                # tail-only inputs, spread into the k-tile stream
                if ko == 40:
                    adjR_sb = pers.tile([128, 2, NV], F32)
                    nc.sync.dma_start(adjR_sb[:, 0, :], adjR[:, 0, :])
                if ko == 46:
                    nc.sync.dma_start(adjR_sb[:, 1, :], adjR[:, 1, :])
                if ko == 58:
                    dvalsb_sb = pers.tile([128, R], F32)
                    nc.sync.dma_start(dvalsb_sb[:], dvalsb[:])
                    biasP_sb = pers.tile([128, 1], F32)
                    nc.sync.dma_start(biasP_sb[:], biasP[:])
                    eye_sb = pers.tile([128, 128], B16)
                    nc.sync.dma_start(eye_sb[:], eye[:])

            # ---- mask with adj_v (diag pre-zeroed on host), cast to bf16 ----
            amk_sb = pers.tile([128, 2, NV], B16)
            for jc in range(4):
                for rh in range(2):
                    cs = slice(jc * 512, (jc + 1) * 512)
                    nc.vector.tensor_tensor(amk_sb[:, rh, cs], psA[rh * 4 + jc][:],
                                            adjR_sb[:, rh, cs],
                                            mybir.AluOpType.mult)

            # ---- transpose + GEMM2: retT[f, r] += G[jt]^T @ amT[jt] ----
            retT_ps = PS.tile([128, 512], mybir.dt.float32, tag="bank",
                              name="retT_ps")
            for jt in range(JT):
                tp = PS.tile([128, 512], B16, tag="bank")
                for rh in range(2):
                    nc.tensor.transpose(tp[:, rh * 128 : (rh + 1) * 128],
                                        amk_sb[:, rh, jt * 128 : (jt + 1) * 128],
                                        eye_sb[:])
                amt = amtp.tile([128, R], B16, tag="amt")
                nc.vector.tensor_copy(amt[:], tp[:, :R])
                nc.tensor.matmul(retT_ps[:, :R], lhsT=G_sb[:, jt, :], rhs=amt[:],
                                 start=(jt == 0), stop=(jt == JT - 1))

            # ---- epilogue: retT + diag(adj_v)*G_rows^T + bias ----
            tmp = pers.tile([128, R], F32, name="tmp")
            nc.vector.tensor_tensor(tmp[:], grT_sb[:], dvalsb_sb[:],
                                    mybir.AluOpType.mult)
            nc.vector.tensor_add(tmp[:], tmp[:], retT_ps[:, :R])
            nc.vector.tensor_scalar_add(tmp[:], tmp[:], biasP_sb[:, 0:1])
            nc.sync.dma_start(out[:], tmp[:])

    nc.finalize()
    return nc


def kernel(H_v, H_e, adj_e, adj_v, T, weight, p, bias):
    global last_exec_time_ns, last_results
    _ensure_ntff_hook()
    from concourse.bass_utils import run_bass_kernel_spmd

    H_v = np.asarray(H_v, np.float32)
    H_e = np.asarray(H_e, np.float32)
    adj_v = np.asarray(adj_v, np.float32)
    T = np.asarray(T, np.float32)
    weight = np.asarray(weight, np.float32)
    p = np.asarray(p, np.float32).reshape(1, IN_E)
    bias = np.asarray(bias, np.float32).reshape(OUT_V)

    if "nc" not in _cache:
        _cache["nc"] = _build()
    nc = _cache["nc"]

    # ---- host-side sharding / layout prep ----
    TTb = np.ascontiguousarray(T.T).astype(BF16)                     # [NE, NV]
    Hen = np.ascontiguousarray(
        H_e.astype(BF16).reshape(KO, 128, IN_E).transpose(1, 0, 2))
    pb = np.ascontiguousarray(
        np.broadcast_to(p.astype(BF16), (128, IN_E)))                # [128, 128]
    A0 = adj_v.copy()
    np.fill_diagonal(A0, 0.0)
    diag = np.ascontiguousarray(np.diag(adj_v))                      # [NV]
    HvT = np.ascontiguousarray(H_v.T.astype(BF16))                   # [128, NV]
    Wb = weight.astype(BF16)
    biasP = np.ascontiguousarray(bias.reshape(OUT_V, 1))
    eye_b = np.eye(128, dtype=BF16)

    in_maps = []
    for c in range(NCORES):
        rows = slice(c * R, (c + 1) * R)
        # roll columns so this core's rows sit at columns [0, R)
        TTc = np.roll(TTb, -c * R, axis=1).reshape(KO, 128, NV)
        adjR_c = np.ascontiguousarray(
            np.roll(A0[rows, :], -c * R, axis=1)
            .reshape(2, 128, NV).transpose(1, 0, 2))                 # [128, 2, NV]
        HvT_c = np.ascontiguousarray(
            np.roll(HvT, -c * R, axis=1).reshape(IN_V, JT, 128))     # [128, JT, 128]
        dvalsb_c = np.ascontiguousarray(
            np.broadcast_to(diag[rows][None, :], (128, R)))          # [128, R]
        HvTr_c = np.ascontiguousarray(HvT[:, rows])                  # [128, R]
        in_maps.append({
            "TTr": TTc, "Hen": Hen, "pb": pb, "adjR": adjR_c,
            "HvT": HvT_c, "HvTr": HvTr_c, "W": Wb,
            "dvalsb": dvalsb_c, "biasP": biasP, "eye": eye_b,
        })

    trace = bool(int(os.environ.get("KERNEL_TRACE", "0")))
    res = run_bass_kernel_spmd(nc, in_maps, list(range(NCORES)), trace=trace)
    last_exec_time_ns = res.exec_time_ns
    last_results = res

    ret = np.concatenate(
        [res.results[c]["out"].T for c in range(NCORES)], axis=0)
    return (ret, H_e)


# revision 15
# speedup vs baseline: 1.0226x; 1.0226x over previous
"""Trainium2 Bass kernel for nn_AlternateConvolution (gnn_message_passing).

Computation (reference):
    w  = (H_e @ p.T)[:, 0]                    # [NE]
    M1 = (T * w) @ T.T                        # [NV, NV]  (symmetric)
    A  = (eye + (1-eye)*M1) * adj_v
    ret = A @ (H_v @ weight) + bias           # [NV, OUT_V]
    return (ret, H_e)

Distribution: output-row shard over 8 NeuronCores (256 rows each); each core
streams the full T^T in bf16 (no collectives). Per-core inputs are column-
rolled so each core's own 256 columns sit at offset 0 of every k-tile; the
scaled stationary operand is sliced straight out of the streamed k-tiles.
The big GEMM keeps the 256-row slab stationary and streams T^T k-tiles
through the moving port (N=512), so LDWEIGHTS traffic stays tiny.
"""

import os
import sys
import types

import numpy as np
import ml_dtypes

NV, NE = 2048, 8192
IN_V, OUT_V, IN_E = 128, 128, 128
NCORES = 8
R = NV // NCORES          # 256 output rows per core
JT = NV // 128            # 16 j-tiles
KO = NE // 128            # 64 k-tiles
HCH = 8                   # chunks for head-critical DMAs / w pipeline
CH = KO // HCH

BF16 = ml_dtypes.bfloat16

_cache = {}
last_exec_time_ns = None
last_results = None


def _ensure_ntff_hook():
    """Register the NTFF profiling hook if the image's antenv lacks it."""
    try:
        import antenv  # noqa: F401
        import antenv.axon_hooks  # noqa: F401
        return
    except ImportError:
        pass
    try:
        import antenv
        from trn_agent_boot.trn_boot import _ntff_profile_via_ctypes

        hook = _ntff_profile_via_ctypes("/opt/axon/libaxon_pjrt.so")
        mod = types.ModuleType("antenv.axon_hooks")
        mod.get_axon_ntff_profile_hook = lambda: hook
        mod.set_axon_ntff_profile_hook = lambda h: None
        sys.modules["antenv.axon_hooks"] = mod
        antenv.axon_hooks = mod
    except Exception:
        pass


def _build():
    import concourse.mybir as mybir
    import concourse.tile as tile
    from concourse import bacc

    F32 = mybir.dt.float32
    B16 = mybir.dt.bfloat16

    nc = bacc.Bacc("TRN2", target_bir_lowering=False, debug=False,
                   num_devices=NCORES)

    # T^T with columns rolled by -256*core: kt[:, 0:R] is this core's slab.
    TTr = nc.declare_dram_parameter("TTr", [KO, 128, NV], B16, isOutput=False)
    Hen = nc.declare_dram_parameter("Hen", [128, KO, 128], B16, isOutput=False)
    pb = nc.declare_dram_parameter("pb", [128, 128], B16, isOutput=False)
    adjR = nc.declare_dram_parameter("adjR", [128, 2, NV], F32, isOutput=False)
    HvT = nc.declare_dram_parameter("HvT", [128, JT, 128], B16, isOutput=False)
    HvTr = nc.declare_dram_parameter("HvTr", [128, R], B16, isOutput=False)
    Wp = nc.declare_dram_parameter("W", [128, 128], B16, isOutput=False)
    dvalsb = nc.declare_dram_parameter("dvalsb", [128, R], F32, isOutput=False)
    biasP = nc.declare_dram_parameter("biasP", [128, 1], F32, isOutput=False)
    eye = nc.declare_dram_parameter("eye", [128, 128], B16, isOutput=False)
    # transposed output: out[f, r] = ret[rows_c[r], f]; host transposes back
    out = nc.declare_dram_parameter("out", [OUT_V, R], F32, isOutput=True)

    with tile.TileContext(nc) as tc:
        with (
            tc.tile_pool(name="pers", bufs=1) as pers,
            tc.tile_pool(name="ktp", bufs=10) as ktp,
            tc.tile_pool(name="amtp", bufs=3) as amtp,
            tc.tile_pool(name="wscr", bufs=3) as wscr,
            tc.tile_pool(name="PS", bufs=8, space="PSUM") as PS,
        ):
            # ---- head-critical DMAs (ScalarE HWDGE queue; Sync is kept
            # exclusively for the k-tile stream) ----
            pb_sb = pers.tile([128, 128], B16)
            nc.scalar.dma_start(pb_sb[:], pb[:])
            Hen_sb = pers.tile([128, KO, 128], B16)
            nc.scalar.dma_start(Hen_sb[:, 0:CH, :], Hen[:, 0:CH, :])
            HvT_sb = pers.tile([128, JT, 128], B16)
            nc.scalar.dma_start(HvT_sb[:], HvT[:])
            HvTr_sb = pers.tile([128, R], B16)
            nc.scalar.dma_start(HvTr_sb[:], HvTr[:])
            W_sb = pers.tile([128, 128], B16)
            nc.scalar.dma_start(W_sb[:], Wp[:])

            w_sb = pers.tile([128, KO], F32)
            G_sb = pers.tile([128, JT, 128], B16)
            scaled_sb = pers.tile([128, KO, R], B16)


            # ---- G = H_v @ weight in the head (warms the PE clock gate) ----
            for jt in range(JT):
                g_ps = PS.tile([128, 512], mybir.dt.float32, tag="bank")
                nc.tensor.matmul(g_ps[:, :128], lhsT=HvT_sb[:, jt, :],
                                 rhs=W_sb[:], start=True, stop=True)
                if jt % 2 == 0:
                    nc.vector.tensor_copy(G_sb[:, jt, :], g_ps[:, :128])
                else:
                    nc.scalar.copy(G_sb[:, jt, :], g_ps[:, :128])
            # G_rows^T for the diagonal term: [f, r]
            grT_sb = pers.tile([128, R], F32)
            grT_ps = PS.tile([128, 512], mybir.dt.float32, tag="bank",
                             name="grT_ps")
            nc.tensor.matmul(grT_ps[:, :R], lhsT=W_sb[:], rhs=HvTr_sb[:],
                             start=True, stop=True)
            nc.vector.tensor_copy(grT_sb[:], grT_ps[:, :R])

            # ---- GEMM1: psA[rh, jc] += scaled[:,ko,rh]^T @ TT[ko][:, jc] ----
            psA = [PS.tile([128, 512], mybir.dt.float32, tag="bank",
                           name=f"psA_{i}") for i in range(8)]
            for ko in range(KO):
                kt = ktp.tile([128, NV], B16, tag="kt")
                nc.sync.dma_start(kt[:], TTr[ko])
                # w[k-tile] on the VectorEngine, just in time for the scale
                wsc = wscr.tile([128, 128], F32, tag="wsc")
                nc.vector.tensor_tensor(wsc[:], Hen_sb[:, ko, :], pb_sb[:],
                                        mybir.AluOpType.mult)
                nc.vector.reduce_sum(w_sb[:, ko : ko + 1], wsc[:],
                                     axis=mybir.AxisListType.X)
                # scaled slab slice straight from the streamed k-tile
                nc.vector.tensor_scalar_mul(scaled_sb[:, ko, :], kt[:, :R],
                                            w_sb[:, ko : ko + 1])
                for rh in range(2):
                    for jc in range(4):
                        nc.tensor.matmul(
                            psA[rh * 4 + jc][:],
                            lhsT=scaled_sb[:, ko, rh * 128 : (rh + 1) * 128],
                            rhs=kt[:, jc * 512 : (jc + 1) * 512],
                            start=(ko == 0), stop=(ko == KO - 1))
                # remaining Hen chunks + tail-only inputs, staggered into
                # the k-tile stream on the ScalarE HWDGE queue
                if ko % CH == 0 and ko // CH < HCH - 1:
                    sl = slice((ko // CH + 1) * CH, (ko // CH + 2) * CH)
                    nc.scalar.dma_start(Hen_sb[:, sl, :], Hen[:, sl, :])
                if ko == 42:
                    adjR_sb = pers.tile([128, 2, NV], F32)
                    nc.scalar.dma_start(adjR_sb[:, 0, :], adjR[:, 0, :])
                if ko == 48:
                    nc.scalar.dma_start(adjR_sb[:, 1, :], adjR[:, 1, :])
                if ko == 54:
                    dvalsb_sb = pers.tile([128, R], F32)
                    nc.scalar.dma_start(dvalsb_sb[:], dvalsb[:])
                    biasP_sb = pers.tile([128, 1], F32)
                    nc.scalar.dma_start(biasP_sb[:], biasP[:])
                    eye_sb = pers.tile([128, 128], B16)
                    nc.scalar.dma_start(eye_sb[:], eye[:])

            # ---- mask with adj_v (diag pre-zeroed on host), cast to bf16 ----
            amk_sb = pers.tile([128, 2, NV], B16)
            for jc in range(4):
                for rh in range(2):
                    cs = slice(jc * 512, (jc + 1) * 512)
                    nc.vector.tensor_tensor(amk_sb[:, rh, cs], psA[rh * 4 + jc][:],
                                            adjR_sb[:, rh, cs],
                                            mybir.AluOpType.mult)

            # ---- transpose + GEMM2: retT[f, r] += G[jt]^T @ amT[jt] ----
            retT_ps = PS.tile([128, 512], mybir.dt.float32, tag="bank",
                              name="retT_ps")
            for jt in range(JT):
                tp = PS.tile([128, 512], B16, tag="bank")
                for rh in range(2):
                    nc.tensor.transpose(tp[:, rh * 128 : (rh + 1) * 128],
                                        amk_sb[:, rh, jt * 128 : (jt + 1) * 128],
                                        eye_sb[:])
                amt = amtp.tile([128, R], B16, tag="amt")
                if jt % 2 == 0:
                    nc.vector.tensor_copy(amt[:], tp[:, :R])
                else:
                    nc.scalar.copy(amt[:], tp[:, :R])
                nc.tensor.matmul(retT_ps[:, :R], lhsT=G_sb[:, jt, :], rhs=amt[:],
                                 start=(jt == 0), stop=(jt == JT - 1))

            # ---- epilogue: retT + diag(adj_v)*G_rows^T + bias ----
            tmp = pers.tile([128, R], F32, name="tmp")
            nc.vector.tensor_tensor(tmp[:], grT_sb[:], dvalsb_sb[:],
                                    mybir.AluOpType.mult)
            nc.vector.tensor_add(tmp[:], tmp[:], retT_ps[:, :R])
            nc.vector.tensor_scalar_add(tmp[:], tmp[:], biasP_sb[:, 0:1])
            nc.sync.dma_start(out[:], tmp[:])

    nc.finalize()
    return nc


def kernel(H_v, H_e, adj_e, adj_v, T, weight, p, bias):
    global last_exec_time_ns, last_results
    _ensure_ntff_hook()
    from concourse.bass_utils import run_bass_kernel_spmd

    H_v = np.asarray(H_v, np.float32)
    H_e = np.asarray(H_e, np.float32)
    adj_v = np.asarray(adj_v, np.float32)
    T = np.asarray(T, np.float32)
    weight = np.asarray(weight, np.float32)
    p = np.asarray(p, np.float32).reshape(1, IN_E)
    bias = np.asarray(bias, np.float32).reshape(OUT_V)

    if "nc" not in _cache:
        _cache["nc"] = _build()
    nc = _cache["nc"]

    # ---- host-side sharding / layout prep ----
    TTb = np.ascontiguousarray(T.T).astype(BF16)                     # [NE, NV]
    Hen = np.ascontiguousarray(
        H_e.astype(BF16).reshape(KO, 128, IN_E).transpose(1, 0, 2))
    pb = np.ascontiguousarray(
        np.broadcast_to(p.astype(BF16), (128, IN_E)))                # [128, 128]
    A0 = adj_v.copy()
    np.fill_diagonal(A0, 0.0)
    diag = np.ascontiguousarray(np.diag(adj_v))                      # [NV]
    HvT = np.ascontiguousarray(H_v.T.astype(BF16))                   # [128, NV]
    Wb = weight.astype(BF16)
    biasP = np.ascontiguousarray(bias.reshape(OUT_V, 1))
    eye_b = np.eye(128, dtype=BF16)

    in_maps = []
    for c in range(NCORES):
        rows = slice(c * R, (c + 1) * R)
        # roll columns so this core's rows sit at columns [0, R)
        TTc = np.roll(TTb, -c * R, axis=1).reshape(KO, 128, NV)
        adjR_c = np.ascontiguousarray(
            np.roll(A0[rows, :], -c * R, axis=1)
            .reshape(2, 128, NV).transpose(1, 0, 2))                 # [128, 2, NV]
        HvT_c = np.ascontiguousarray(
            np.roll(HvT, -c * R, axis=1).reshape(IN_V, JT, 128))     # [128, JT, 128]
        dvalsb_c = np.ascontiguousarray(
            np.broadcast_to(diag[rows][None, :], (128, R)))          # [128, R]
        HvTr_c = np.ascontiguousarray(HvT[:, rows])                  # [128, R]
        in_maps.append({
            "TTr": TTc, "Hen": Hen, "pb": pb, "adjR": adjR_c,
            "HvT": HvT_c, "HvTr": HvTr_c, "W": Wb,
            "dvalsb": dvalsb_c, "biasP": biasP, "eye": eye_b,
        })

    trace = bool(int(os.environ.get("KERNEL_TRACE", "0")))
    res = run_bass_kernel_spmd(nc, in_maps, list(range(NCORES)), trace=trace)
    last_exec_time_ns = res.exec_time_ns
    last_results = res

    ret = np.concatenate(
        [res.results[c]["out"].T for c in range(NCORES)], axis=0)
    return (ret, H_e)


# revision 16
# speedup vs baseline: 1.1850x; 1.1588x over previous
"""Trainium2 Bass kernel for nn_AlternateConvolution (gnn_message_passing).

Computation (reference):
    w  = (H_e @ p.T)[:, 0]                    # [NE]
    M1 = (T * w) @ T.T                        # [NV, NV]  (symmetric)
    A  = (eye + (1-eye)*M1) * adj_v
    ret = A @ (H_v @ weight) + bias           # [NV, OUT_V]
    return (ret, H_e)

Distribution: output-row shard over 8 NeuronCores (256 rows each); each core
streams the full T^T in bf16 (no collectives). Per-core inputs are column-
rolled so each core's own 256 columns sit at offset 0 of every k-tile; the
scaled stationary operand is sliced straight out of the streamed k-tiles.
The big GEMM keeps the 256-row slab stationary and streams T^T k-tiles
through the moving port (N=512), so LDWEIGHTS traffic stays tiny.
"""

import os
import sys
import types

import numpy as np
import ml_dtypes

NV, NE = 2048, 8192
IN_V, OUT_V, IN_E = 128, 128, 128
NCORES = 8
R = NV // NCORES          # 256 output rows per core
JT = NV // 128            # 16 j-tiles
KO = NE // 128            # 64 k-tiles
HCH = 8                   # chunks for head-critical DMAs / w pipeline
CH = KO // HCH

BF16 = ml_dtypes.bfloat16

_cache = {}
last_exec_time_ns = None
last_results = None


def _ensure_ntff_hook():
    """Register the NTFF profiling hook if the image's antenv lacks it."""
    try:
        import antenv  # noqa: F401
        import antenv.axon_hooks  # noqa: F401
        return
    except ImportError:
        pass
    try:
        import antenv
        from trn_agent_boot.trn_boot import _ntff_profile_via_ctypes

        hook = _ntff_profile_via_ctypes("/opt/axon/libaxon_pjrt.so")
        mod = types.ModuleType("antenv.axon_hooks")
        mod.get_axon_ntff_profile_hook = lambda: hook
        mod.set_axon_ntff_profile_hook = lambda h: None
        sys.modules["antenv.axon_hooks"] = mod
        antenv.axon_hooks = mod
    except Exception:
        pass


def _build():
    import concourse.mybir as mybir
    import concourse.tile as tile
    from concourse import bacc

    F32 = mybir.dt.float32
    B16 = mybir.dt.bfloat16

    nc = bacc.Bacc("TRN2", target_bir_lowering=False, debug=False,
                   num_devices=NCORES)

    # T^T with columns rolled by -256*core: kt[:, 0:R] is this core's slab.
    TTr = nc.declare_dram_parameter("TTr", [KO, 128, NV], B16, isOutput=False)
    Hen = nc.declare_dram_parameter("Hen", [128, KO, 128], B16, isOutput=False)
    pb = nc.declare_dram_parameter("pb", [128, 128], B16, isOutput=False)
    adjR = nc.declare_dram_parameter("adjR", [128, 2, NV], F32, isOutput=False)
    HvT = nc.declare_dram_parameter("HvT", [128, JT, 128], B16, isOutput=False)
    HvTr = nc.declare_dram_parameter("HvTr", [128, R], B16, isOutput=False)
    Wp = nc.declare_dram_parameter("W", [128, 128], B16, isOutput=False)
    dvalsb = nc.declare_dram_parameter("dvalsb", [128, R], F32, isOutput=False)
    biasP = nc.declare_dram_parameter("biasP", [128, 1], F32, isOutput=False)
    eye = nc.declare_dram_parameter("eye", [128, 128], B16, isOutput=False)
    # transposed output: out[f, r] = ret[rows_c[r], f]; host transposes back
    out = nc.declare_dram_parameter("out", [OUT_V, R], F32, isOutput=True)

    with tile.TileContext(nc) as tc:
        with (
            tc.tile_pool(name="pers", bufs=1) as pers,
            tc.tile_pool(name="ktp", bufs=10) as ktp,
            tc.tile_pool(name="amtp", bufs=3) as amtp,
            tc.tile_pool(name="wscr", bufs=3) as wscr,
            tc.tile_pool(name="PS", bufs=8, space="PSUM") as PS,
        ):
            # ---- head-critical DMAs: issued on the Sync queue BEFORE the
            # k-tile stream so their completions aren't gated behind k-tile
            # transfers on the shared HWDGE semaphore lanes ----
            pb_sb = pers.tile([128, 128], B16)
            nc.sync.dma_start(pb_sb[:], pb[:])
            HvT_sb = pers.tile([128, JT, 128], B16)
            nc.sync.dma_start(HvT_sb[:], HvT[:])
            W_sb = pers.tile([128, 128], B16)
            nc.sync.dma_start(W_sb[:], Wp[:])
            HvTr_sb = pers.tile([128, R], B16)
            nc.sync.dma_start(HvTr_sb[:], HvTr[:])
            Hen_sb = pers.tile([128, KO, 128], B16)
            nc.sync.dma_start(Hen_sb[:, 0:CH, :], Hen[:, 0:CH, :])

            w_sb = pers.tile([128, KO], F32)
            G_sb = pers.tile([128, JT, 128], B16)
            scaled_sb = pers.tile([128, KO, R], B16)


            # ---- G = H_v @ weight in the head (warms the PE clock gate) ----
            for jt in range(JT):
                g_ps = PS.tile([128, 512], mybir.dt.float32, tag="bank")
                nc.tensor.matmul(g_ps[:, :128], lhsT=HvT_sb[:, jt, :],
                                 rhs=W_sb[:], start=True, stop=True)
                if jt % 2 == 0:
                    nc.vector.tensor_copy(G_sb[:, jt, :], g_ps[:, :128])
                else:
                    nc.scalar.copy(G_sb[:, jt, :], g_ps[:, :128])
            # G_rows^T for the diagonal term: [f, r]
            grT_sb = pers.tile([128, R], F32)
            grT_ps = PS.tile([128, 512], mybir.dt.float32, tag="bank",
                             name="grT_ps")
            nc.tensor.matmul(grT_ps[:, :R], lhsT=W_sb[:], rhs=HvTr_sb[:],
                             start=True, stop=True)
            nc.vector.tensor_copy(grT_sb[:], grT_ps[:, :R])

            # ---- GEMM1: psA[rh, jc] += scaled[:,ko,rh]^T @ TT[ko][:, jc] ----
            psA = [PS.tile([128, 512], mybir.dt.float32, tag="bank",
                           name=f"psA_{i}") for i in range(8)]
            for ko in range(KO):
                kt = ktp.tile([128, NV], B16, tag="kt")
                nc.sync.dma_start(kt[:], TTr[ko])
                # w[k-tile] on the VectorEngine, just in time for the scale
                wsc = wscr.tile([128, 128], F32, tag="wsc")
                nc.vector.tensor_tensor(wsc[:], Hen_sb[:, ko, :], pb_sb[:],
                                        mybir.AluOpType.mult)
                nc.vector.reduce_sum(w_sb[:, ko : ko + 1], wsc[:],
                                     axis=mybir.AxisListType.X)
                # scaled slab slice straight from the streamed k-tile
                nc.vector.tensor_scalar_mul(scaled_sb[:, ko, :], kt[:, :R],
                                            w_sb[:, ko : ko + 1])
                for rh in range(2):
                    for jc in range(4):
                        nc.tensor.matmul(
                            psA[rh * 4 + jc][:],
                            lhsT=scaled_sb[:, ko, rh * 128 : (rh + 1) * 128],
                            rhs=kt[:, jc * 512 : (jc + 1) * 512],
                            start=(ko == 0), stop=(ko == KO - 1))
                # remaining Hen chunks + tail-only inputs, staggered into
                # the k-tile stream on the ScalarE HWDGE queue
                if ko % CH == 0 and ko // CH < HCH - 1:
                    sl = slice((ko // CH + 1) * CH, (ko // CH + 2) * CH)
                    nc.scalar.dma_start(Hen_sb[:, sl, :], Hen[:, sl, :])
                if ko == 42:
                    adjR_sb = pers.tile([128, 2, NV], F32)
                    nc.scalar.dma_start(adjR_sb[:, 0, :], adjR[:, 0, :])
                if ko == 48:
                    nc.scalar.dma_start(adjR_sb[:, 1, :], adjR[:, 1, :])
                if ko == 54:
                    dvalsb_sb = pers.tile([128, R], F32)
                    nc.scalar.dma_start(dvalsb_sb[:], dvalsb[:])
                    biasP_sb = pers.tile([128, 1], F32)
                    nc.scalar.dma_start(biasP_sb[:], biasP[:])
                    eye_sb = pers.tile([128, 128], B16)
                    nc.scalar.dma_start(eye_sb[:], eye[:])

            # ---- mask with adj_v (diag pre-zeroed on host), cast to bf16 ----
            amk_sb = pers.tile([128, 2, NV], B16)
            for jc in range(4):
                for rh in range(2):
                    cs = slice(jc * 512, (jc + 1) * 512)
                    nc.vector.tensor_tensor(amk_sb[:, rh, cs], psA[rh * 4 + jc][:],
                                            adjR_sb[:, rh, cs],
                                            mybir.AluOpType.mult)

            # ---- transpose + GEMM2: retT[f, r] += G[jt]^T @ amT[jt] ----
            retT_ps = PS.tile([128, 512], mybir.dt.float32, tag="bank",
                              name="retT_ps")
            for jt in range(JT):
                tp = PS.tile([128, 512], B16, tag="bank")
                for rh in range(2):
                    nc.tensor.transpose(tp[:, rh * 128 : (rh + 1) * 128],
                                        amk_sb[:, rh, jt * 128 : (jt + 1) * 128],
                                        eye_sb[:])
                amt = amtp.tile([128, R], B16, tag="amt")
                if jt % 2 == 0:
                    nc.vector.tensor_copy(amt[:], tp[:, :R])
                else:
                    nc.scalar.copy(amt[:], tp[:, :R])
                nc.tensor.matmul(retT_ps[:, :R], lhsT=G_sb[:, jt, :], rhs=amt[:],
                                 start=(jt == 0), stop=(jt == JT - 1))

            # ---- epilogue: retT + diag(adj_v)*G_rows^T + bias ----
            tmp = pers.tile([128, R], F32, name="tmp")
            nc.vector.tensor_tensor(tmp[:], grT_sb[:], dvalsb_sb[:],
                                    mybir.AluOpType.mult)
            nc.vector.tensor_add(tmp[:], tmp[:], retT_ps[:, :R])
            nc.vector.tensor_scalar_add(tmp[:], tmp[:], biasP_sb[:, 0:1])
            nc.sync.dma_start(out[:], tmp[:])

    nc.finalize()
    return nc


def kernel(H_v, H_e, adj_e, adj_v, T, weight, p, bias):
    global last_exec_time_ns, last_results
    _ensure_ntff_hook()
    from concourse.bass_utils import run_bass_kernel_spmd

    H_v = np.asarray(H_v, np.float32)
    H_e = np.asarray(H_e, np.float32)
    adj_v = np.asarray(adj_v, np.float32)
    T = np.asarray(T, np.float32)
    weight = np.asarray(weight, np.float32)
    p = np.asarray(p, np.float32).reshape(1, IN_E)
    bias = np.asarray(bias, np.float32).reshape(OUT_V)

    if "nc" not in _cache:
        _cache["nc"] = _build()
    nc = _cache["nc"]

    # ---- host-side sharding / layout prep ----
    TTb = np.ascontiguousarray(T.T).astype(BF16)                     # [NE, NV]
    Hen = np.ascontiguousarray(
        H_e.astype(BF16).reshape(KO, 128, IN_E).transpose(1, 0, 2))
    pb = np.ascontiguousarray(
        np.broadcast_to(p.astype(BF16), (128, IN_E)))                # [128, 128]
    A0 = adj_v.copy()
    np.fill_diagonal(A0, 0.0)
    diag = np.ascontiguousarray(np.diag(adj_v))                      # [NV]
    HvT = np.ascontiguousarray(H_v.T.astype(BF16))                   # [128, NV]
    Wb = weight.astype(BF16)
    biasP = np.ascontiguousarray(bias.reshape(OUT_V, 1))
    eye_b = np.eye(128, dtype=BF16)

    in_maps = []
    for c in range(NCORES):
        rows = slice(c * R, (c + 1) * R)
        # roll columns so this core's rows sit at columns [0, R)
        TTc = np.roll(TTb, -c * R, axis=1).reshape(KO, 128, NV)
        adjR_c = np.ascontiguousarray(
            np.roll(A0[rows, :], -c * R, axis=1)
            .reshape(2, 128, NV).transpose(1, 0, 2))                 # [128, 2, NV]
        HvT_c = np.ascontiguousarray(
            np.roll(HvT, -c * R, axis=1).reshape(IN_V, JT, 128))     # [128, JT, 128]
        dvalsb_c = np.ascontiguousarray(
            np.broadcast_to(diag[rows][None, :], (128, R)))          # [128, R]
        HvTr_c = np.ascontiguousarray(HvT[:, rows])                  # [128, R]
        in_maps.append({
            "TTr": TTc, "Hen": Hen, "pb": pb, "adjR": adjR_c,
            "HvT": HvT_c, "HvTr": HvTr_c, "W": Wb,
            "dvalsb": dvalsb_c, "biasP": biasP, "eye": eye_b,
        })

    trace = bool(int(os.environ.get("KERNEL_TRACE", "0")))
    res = run_bass_kernel_spmd(nc, in_maps, list(range(NCORES)), trace=trace)
    last_exec_time_ns = res.exec_time_ns
    last_results = res

    ret = np.concatenate(
        [res.results[c]["out"].T for c in range(NCORES)], axis=0)
    return (ret, H_e)
